# revision 3
# baseline (speedup 1.0000x reference)
"""Trainium2 kernel for nn_HadamardRotation: y = x @ H, H = 4096x4096 Walsh-Hadamard.

Strategy
--------
H4096 = H64 (x) H64 (Kronecker). Writing d = 64*hi + lo, e = 64*hi' + lo':

    y[r, e] = sum_{hi,lo} H64[lo,lo'] * H64[hi,hi'] * x[r, d]

Two matmul stages with 128-wide contraction (block-diagonal I2 (x) H64
weights), separated by an on-chip "corner turn" (SBUF->SBUF DMA partition
shuffle), all operating in the transposed domain (d on partitions, rows on
the free axis). Host does the cheap transposes / index unscrambles; the
device only ever issues contiguous >=1KB DMA lines.

FLOPs: 2 * 128/4096 of the naive matmul = 16x reduction.

Data parallel over 8 cores: rows sharded 16384 -> 8 x 2048, weights
replicated.

Layouts (per core, R = 2048 rows):
  xt  DRAM in  (32, 128, R): xt[a, 64*mu+lo, r] = x[r, 128*a + 64*mu + lo]
  B1  (128,128): B1[64*mu+lo, 2*lo'+mu]     = H64[lo, lo']
  B2  (128,128): B2[64*nu+32*mu+a, 2*hi'+nu] = H64[2*a+mu, hi']
  stage A (chunk a): u_a[p, r] = sum_k B1[k, p] xt[a, k, r]
      => u_a[4c + (2*nu+mu)] holds (hi = 2a+mu, lo' = 2c+nu)
  corner turn:  v_c[32*t + a, r] = u_a[4*c + t, r]
  stage B (chunk c): Y[c, m, r] = sum_q B2[q, m] v_c[q, r]
      => Y[c, 2*hi'+nu, r] = y[r, 64*hi' + 2*c + nu]
"""

import math
import numpy as np
import ml_dtypes

import concourse.bass as bass
import concourse.mybir as mybir
import concourse.tile as tile
from concourse import bacc
from concourse.bass_utils import run_bass_kernel_spmd

N_CORES = 8
DIM = 4096
R_TOTAL = 4 * 4096          # rows after flattening (4, 4096, DIM)
R = R_TOTAL // N_CORES      # rows per core
N = 512                     # free-dim slab (one PSUM bank of fp32)
SLABS = R // N

# dtype mode: "fp32" (exact, PE 4 cyc/row), "fp32r" (fp32 storage, fast PE
# mode), "bf16" (half storage+DMA for x/intermediate, exact weights)
MODE = "bf16"

# tuning knobs (overridable for benching)
CFG = dict(
    ycopy="vector2",   # engine for psum->sbuf copy of stage-B out: vector|any|vector2 (split DVE/ACT)
    ucopy="vector",    # engine for psum->sbuf copy of stage-A out
    turn_eng="scalar",  # corner-turn DMA engine: scalar|sync|gpsimd|rr (round robin)
    in_eng="sync",
    out_eng="sync",
    in_batch=4,        # chunks per input DMA
    out_batch=4,       # batch output DMAs over this many c-chunks
    turn_slabs=1,      # how many N-slabs share one corner-turn DMA
    pipeline=1,        # emit stage A of slab s+1 before stage B of slab s
    ycast=0,           # stage-B out staged as bf16 in SBUF, SWDGE casts to f32
    xbufs=3, ubufs=2, vbufs=4, ybufs=4,
)


def _walsh_hadamard64():
    h = np.array([[1.0]], dtype=np.float64)
    while h.shape[0] < 64:
        h = np.block([[h, h], [h, -h]]) / math.sqrt(2.0)
    return h.astype(np.float32)


def _build_weights(H64):
    B1 = np.zeros((128, 128), dtype=np.float32)
    b1v = B1.reshape(2, 64, 64, 2)
    for mu in range(2):
        b1v[mu, :, :, mu] = H64
    B2 = np.zeros((128, 128), dtype=np.float32)
    b2v = B2.reshape(2, 2, 32, 64, 2)
    for nu in range(2):
        for mu in range(2):
            b2v[nu, mu, :, :, nu] = H64[mu::2, :]
    return B1, B2


_NC_CACHE = {}


def _build_bass(mode, loop=0, cfg=None):
    cfg = dict(CFG, **(cfg or {}))
    key = (mode, loop, tuple(sorted(cfg.items())))
    if key in _NC_CACHE:
        return _NC_CACHE[key]

    f32 = mybir.dt.float32
    dt_in = mybir.dt.bfloat16 if mode == "bf16" else f32
    mm_cast = (lambda ap: ap.bitcast(mybir.dt.float32r)) if mode == "fp32r" else (lambda ap: ap)

    nc = bacc.Bacc("TRN2", target_bir_lowering=False, debug=False,
                   num_devices=N_CORES)
    xt_d = nc.dram_tensor("xt", [32, 128, R], dt_in, kind="ExternalInput")
    B1_d = nc.dram_tensor("B1", [128, 128], dt_in, kind="ExternalInput")
    B2_d = nc.dram_tensor("B2", [128, 128], dt_in, kind="ExternalInput")
    Y_d = nc.dram_tensor("Y", [32, 128, R], f32, kind="ExternalOutput")

    OB = cfg["out_batch"]

    with tile.TileContext(nc) as tc:
        with (
            tc.tile_pool(name="wpool", bufs=1) as wpool,
            tc.tile_pool(name="xpool", bufs=cfg["xbufs"]) as xpool,
            tc.tile_pool(name="upool", bufs=cfg["ubufs"]) as upool,
            tc.tile_pool(name="vpool", bufs=cfg["vbufs"]) as vpool,
            tc.tile_pool(name="ypool", bufs=cfg["ybufs"]) as ypool,
            tc.tile_pool(name="psA", bufs=4, space="PSUM") as psA,
            tc.tile_pool(name="psB", bufs=4, space="PSUM") as psB,
        ):
            B1_sb = wpool.tile([128, 128], dt_in)
            nc.sync.dma_start(B1_sb[:], B1_d[:])
            B2_sb = wpool.tile([128, 128], dt_in)
            nc.sync.dma_start(B2_sb[:], B2_d[:])

            in_eng = getattr(nc, cfg["in_eng"])
            out_eng = getattr(nc, cfg["out_eng"])
            turn_eng = None if cfg["turn_eng"] == "rr" else getattr(nc, cfg["turn_eng"])

            def copy(engine, dst, src, i):
                if engine == "vector":
                    nc.vector.tensor_copy(dst, src)
                elif engine == "vector2":
                    # alternate DVE / ACT so neither engine binds
                    if i % 2 == 0:
                        nc.vector.tensor_copy(dst, src)
                    else:
                        nc.any.tensor_copy(dst, src)
                else:
                    nc.any.tensor_copy(dst, src)

            turn_rr = [nc.scalar, nc.sync, nc.gpsimd]

            def turn(i):
                if cfg["turn_eng"] == "rr":
                    return turn_rr[i % 3]
                return turn_eng

            TS = cfg["turn_slabs"]
            IB = cfg["in_batch"]

            def phaseA(sg):
                    u_all = upool.tile([128, 32, TS * N], dt_in)
                    for ts in range(TS):
                        s = sg * TS + ts
                        ns = slice(s * N, (s + 1) * N)
                        for g in range(32 // IB):
                            xg = xpool.tile([128, IB, N], dt_in)
                            in_eng.dma_start(
                                xg[:],
                                xt_d[IB * g:IB * (g + 1), :, ns].transpose([1, 0, 2]))
                            for j in range(IB):
                                a = IB * g + j
                                pu = psA.tile([128, N], f32)
                                nc.tensor.matmul(pu[:], mm_cast(B1_sb[:]),
                                                 mm_cast(xg[:, j, :]),
                                                 start=True, stop=True)
                                copy(cfg["ucopy"],
                                     u_all[:, a, ts * N:(ts + 1) * N], pu[:], a)
                    return u_all

            def phaseB(sg, u_all):
                    # corner turn + stage B
                    ut = u_all.tensor
                    PU = u_all.ap[0][0]  # partition stride in elements
                    L = TS * N
                    dt_y = mybir.dt.bfloat16 if cfg["ycast"] else f32
                    y_eng = nc.gpsimd if cfg["ycast"] else out_eng
                    for cb in range(32 // OB):
                        ybs = [ypool.tile([128, OB, N], dt_y, name=f"yb{ts}")
                               for ts in range(TS)]
                        for j in range(OB):
                            c = cb * OB + j
                            vc = vpool.tile([128, L], dt_in)
                            in_ap = bass.AP(ut, 4 * c * PU,
                                            [[PU, 4], [L, 32], [1, L]])
                            turn(c).dma_start(vc[:], in_ap)
                            for ts in range(TS):
                                py = psB.tile([128, N], f32)
                                nc.tensor.matmul(py[:], mm_cast(B2_sb[:]),
                                                 mm_cast(vc[:, ts * N:(ts + 1) * N]),
                                                 start=True, stop=True)
                                copy(cfg["ycopy"], ybs[ts][:, j, :], py[:], c + ts)
                        for ts in range(TS):
                            s = sg * TS + ts
                            y_eng.dma_start(
                                Y_d[cb * OB:(cb + 1) * OB, :,
                                    s * N:(s + 1) * N].transpose([1, 0, 2]),
                                ybs[ts][:])

            def body():
                if cfg["pipeline"]:
                    # software pipeline: emit stage A of slab-group sg+1
                    # before stage B of sg, so PE never stalls on the turn.
                    pending = None
                    for sg in range(SLABS // TS):
                        u_all = phaseA(sg)
                        if pending is not None:
                            phaseB(*pending)
                        pending = (sg, u_all)
                    phaseB(*pending)
                else:
                    for sg in range(SLABS // TS):
                        phaseB(sg, phaseA(sg))

            if loop:
                with tc.For_i(0, loop, 1):
                    body()
            else:
                body()

    nc.compile()
    _NC_CACHE[key] = nc
    return nc


def _prep_inputs(x, H, mode):
    np_in = ml_dtypes.bfloat16 if mode == "bf16" else np.float32
    H64 = (np.asarray(H, dtype=np.float32)[::64, ::64] * 8.0).astype(np.float32)
    B1, B2 = _build_weights(H64)
    B1 = B1.astype(np_in)
    B2 = B2.astype(np_in)
    xf = np.asarray(x, dtype=np.float32).reshape(R_TOTAL, DIM)
    in_maps = []
    for i in range(N_CORES):
        shard = xf[i * R:(i + 1) * R]                     # (R, DIM)
        xt = np.ascontiguousarray(shard.T, dtype=np_in)   # (DIM, R)
        xt = xt.reshape(32, 128, R)
        in_maps.append({"xt": xt, "B1": B1, "B2": B2})
    return in_maps


def _unscramble(results):
    outs = []
    for i in range(N_CORES):
        Y = results[i]["Y"]                               # (32, 128, R) f32
        y = Y.reshape(32, 64, 2, R).transpose(3, 1, 0, 2).reshape(R, DIM)
        outs.append(y)
    return np.concatenate(outs, axis=0).reshape(4, 4096, DIM).astype(np.float32)


def _install_ntff_hook():
    """Provide antenv.axon_hooks.get_axon_ntff_profile_hook via ctypes on
    the baked libaxon_pjrt.so (the agent image lacks the module). Only used
    on the _trace path."""
    import sys, types, ctypes, contextlib
    if "antenv.axon_hooks" in sys.modules:
        return
    try:
        lib = ctypes.CDLL("/opt/axon/libaxon_pjrt.so")
        if not hasattr(lib, "axon_start_nrt_profile"):
            return
    except OSError:
        return
    lib.axon_start_nrt_profile.argtypes = [ctypes.POINTER(ctypes.c_int64),
                                           ctypes.c_size_t]
    lib.axon_start_nrt_profile.restype = ctypes.c_int64
    lib.axon_stop_nrt_profile.argtypes = [ctypes.c_char_p]
    lib.axon_stop_nrt_profile.restype = ctypes.c_int64

    @contextlib.contextmanager
    def _hook(output_dir, device_ids):
        import jax
        jax.devices()
        if device_ids:
            ids = (ctypes.c_int64 * len(device_ids))(*device_ids)
            rc = lib.axon_start_nrt_profile(ids, len(device_ids))
        else:
            rc = lib.axon_start_nrt_profile(None, 0)
        if rc != 0:
            raise RuntimeError(f"axon_start_nrt_profile rc={rc}")
        try:
            yield
        finally:
            n = lib.axon_stop_nrt_profile(str(output_dir).encode())
            print(f"ntff profile: {n} file(s) -> {output_dir}")

    mod = types.ModuleType("antenv.axon_hooks")
    mod.get_axon_ntff_profile_hook = lambda: _hook
    sys.modules["antenv.axon_hooks"] = mod


def kernel(x, H, _trace=False, _loop=0, _cfg=None):
    if _trace:
        _install_ntff_hook()
        from concourse import bass_utils as _bu
        _bu.upload_artifacts = lambda d: d
    nc = _build_bass(MODE, loop=_loop, cfg=_cfg)
    in_maps = _prep_inputs(x, H, MODE)
    res = run_bass_kernel_spmd(nc, in_maps, core_ids=list(range(N_CORES)),
                               trace=_trace)
    out = _unscramble(res.results)
    if _trace:
        return out, res
    return out



# revision 4
# speedup vs baseline: 1.0081x; 1.0081x over previous
"""Trainium2 kernel for nn_HadamardRotation: y = x @ H, H = 4096x4096 Walsh-Hadamard.

Strategy
--------
H4096 = H64 (x) H64 (Kronecker). Writing d = 64*hi + lo, e = 64*hi' + lo':

    y[r, e] = sum_{hi,lo} H64[lo,lo'] * H64[hi,hi'] * x[r, d]

Two matmul stages with 128-wide contraction (block-diagonal I2 (x) H64
weights), separated by an on-chip "corner turn" (SBUF->SBUF DMA partition
shuffle), all operating in the transposed domain (d on partitions, rows on
the free axis). Host does the cheap transposes / index unscrambles; the
device only ever issues contiguous 4KB DMA lines.

FLOPs: 2 * 128/4096 of the naive matmul = 16x reduction.

Data parallel over 8 cores: rows sharded 16384 -> 8 x 2048, weights
replicated.

Layouts (per core, R = 2048 rows):
  xt  DRAM in  (32, 128, R): xt[a, 64*mu+lo, r] = x[r, 128*a + 64*mu + lo]
  B1  (128,128): B1[64*mu+lo, 2*lo'+mu]     = H64[lo, lo']
  B2  (128,128): B2[64*nu+32*mu+a, 2*hi'+nu] = H64[2*a+mu, hi']
  stage A (chunk a): u[p, a, r] = sum_k B1[k, p] xt[a, k, r]
      => u[4c + (2*nu+mu), a] holds (hi = 2a+mu, lo' = 2c+nu)
  corner turn:  v_c[32*t + a, r] = u[4*c + t, a, r]
  stage B (chunk c): Y[c, m, r] = sum_q B2[q, m] v_c[q, r]
      => Y[c, 2*hi'+nu, r] = y[r, 64*hi' + 2*c + nu]

Perf notes (from NTFF profiles): the kernel is DMA-bound. All bulk DMA
(input load, corner turn, output store) is issued from the SP engine so it
lands on qSyncDynamicHW, the only queue striped over all 16 DMA engines
(the scalar/ACT HWDGE queue only gets 4). Every transfer moves 4KB
contiguous lines. Output is written bf16 (host upcasts) to halve the HBM
write. Total DMA = 3 x 16.8MB per core ~= 50MB at 360 GB/s aggregate.
"""

import math
import numpy as np
import ml_dtypes

import concourse.bass as bass
import concourse.mybir as mybir
import concourse.tile as tile
from concourse import bacc
from concourse.bass_utils import run_bass_kernel_spmd

N_CORES = 8
DIM = 4096
R_TOTAL = 4 * 4096          # rows after flattening (4, 4096, DIM)
R = R_TOTAL // N_CORES      # rows per core
N = 512                     # matmul free-dim slab (one PSUM bank of fp32)
NSLICE = R // N             # matmul slices per chunk

MODE = "bf16"

CFG = dict(
    in_eng="sync",
    turn_eng="sync",
    out_eng="sync",
    copy_rot=("vector", "any"),  # engines for psum->sbuf copies, round robin
    xbufs=3, vbufs=3, ybufs=3,
    lookahead=2,                 # turn DMAs in flight ahead of stage-B use
)


def _walsh_hadamard64():
    h = np.array([[1.0]], dtype=np.float64)
    while h.shape[0] < 64:
        h = np.block([[h, h], [h, -h]]) / math.sqrt(2.0)
    return h.astype(np.float32)


def _build_weights(H64):
    B1 = np.zeros((128, 128), dtype=np.float32)
    b1v = B1.reshape(2, 64, 64, 2)
    for mu in range(2):
        b1v[mu, :, :, mu] = H64
    B2 = np.zeros((128, 128), dtype=np.float32)
    b2v = B2.reshape(2, 2, 32, 64, 2)
    for nu in range(2):
        for mu in range(2):
            b2v[nu, mu, :, :, nu] = H64[mu::2, :]
    return B1, B2


_NC_CACHE = {}


def _build_bass(mode, loop=0, cfg=None):
    cfg = dict(CFG, **(cfg or {}))
    key = (mode, loop, tuple(sorted((k, str(v)) for k, v in cfg.items())))
    if key in _NC_CACHE:
        return _NC_CACHE[key]

    f32 = mybir.dt.float32
    dt_in = mybir.dt.bfloat16 if mode == "bf16" else f32
    dt_out = mybir.dt.bfloat16 if mode == "bf16" else f32

    nc = bacc.Bacc("TRN2", target_bir_lowering=False, debug=False,
                   num_devices=N_CORES)
    xt_d = nc.dram_tensor("xt", [32, 128, R], dt_in, kind="ExternalInput")
    B1_d = nc.dram_tensor("B1", [128, 128], dt_in, kind="ExternalInput")
    B2_d = nc.dram_tensor("B2", [128, 128], dt_in, kind="ExternalInput")
    Y_d = nc.dram_tensor("Y", [32, 128, R], dt_out, kind="ExternalOutput")

    with tile.TileContext(nc) as tc:
        with (
            tc.tile_pool(name="wpool", bufs=1) as wpool,
            tc.tile_pool(name="xpool", bufs=cfg["xbufs"]) as xpool,
            tc.tile_pool(name="upool", bufs=1) as upool,
            tc.tile_pool(name="vpool", bufs=cfg["vbufs"]) as vpool,
            tc.tile_pool(name="ypool", bufs=cfg["ybufs"]) as ypool,
            tc.tile_pool(name="psA", bufs=4, space="PSUM") as psA,
            tc.tile_pool(name="psB", bufs=4, space="PSUM") as psB,
        ):
            B1_sb = wpool.tile([128, 128], dt_in)
            nc.sync.dma_start(B1_sb[:], B1_d[:])
            B2_sb = wpool.tile([128, 128], dt_in)
            nc.sync.dma_start(B2_sb[:], B2_d[:])

            in_eng = getattr(nc, cfg["in_eng"])
            turn_eng = getattr(nc, cfg["turn_eng"])
            out_eng = getattr(nc, cfg["out_eng"])
            rot = cfg["copy_rot"]

            def copy(dst, src, i):
                getattr(nc, rot[i % len(rot)]).tensor_copy(dst, src)

            def body():
                u_full = upool.tile([128, 32, R], dt_in)

                # ---- phase A: load chunks, stage-A matmuls into u ----
                ci = 0
                for a in range(32):
                    xg = xpool.tile([128, R], dt_in)
                    in_eng.dma_start(xg[:], xt_d[a])
                    for s in range(NSLICE):
                        pu = psA.tile([128, N], f32)
                        nc.tensor.matmul(pu[:], B1_sb[:],
                                         xg[:, s * N:(s + 1) * N],
                                         start=True, stop=True)
                        copy(u_full[:, a, s * N:(s + 1) * N], pu[:], ci)
                        ci += 1

                # ---- phase B: corner turn + stage-B matmuls + store ----
                PU = u_full.ap[0][0]  # partition stride in elements

                def turn(c):
                    vc = vpool.tile([128, R], dt_in)
                    src = bass.AP(u_full.tensor, 4 * c * PU,
                                  [[PU, 4], [R, 32], [1, R]])
                    turn_eng.dma_start(vc[:], src)
                    return vc

                def stageB(c, vc, ci):
                    yb = ypool.tile([128, R], dt_out)
                    for s in range(NSLICE):
                        py = psB.tile([128, N], f32)
                        nc.tensor.matmul(py[:], B2_sb[:],
                                         vc[:, s * N:(s + 1) * N],
                                         start=True, stop=True)
                        copy(yb[:, s * N:(s + 1) * N], py[:], ci + s)
                    out_eng.dma_start(Y_d[c], yb[:])

                pending = []
                for c in range(32):
                    pending.append((c, turn(c)))
                    if len(pending) > cfg["lookahead"]:
                        pc, pv = pending.pop(0)
                        stageB(pc, pv, ci)
                        ci += NSLICE
                for pc, pv in pending:
                    stageB(pc, pv, ci)
                    ci += NSLICE

            if loop:
                with tc.For_i(0, loop, 1):
                    body()
            else:
                body()

    nc.compile()
    _NC_CACHE[key] = nc
    return nc


def _prep_inputs(x, H, mode):
    np_in = ml_dtypes.bfloat16 if mode == "bf16" else np.float32
    H64 = (np.asarray(H, dtype=np.float32)[::64, ::64] * 8.0).astype(np.float32)
    B1, B2 = _build_weights(H64)
    B1 = B1.astype(np_in)
    B2 = B2.astype(np_in)
    xf = np.asarray(x, dtype=np.float32).reshape(R_TOTAL, DIM)
    in_maps = []
    for i in range(N_CORES):
        shard = xf[i * R:(i + 1) * R]                     # (R, DIM)
        xt = np.ascontiguousarray(shard.T, dtype=np_in)   # (DIM, R)
        xt = xt.reshape(32, 128, R)
        in_maps.append({"xt": xt, "B1": B1, "B2": B2})
    return in_maps


def _unscramble(results):
    outs = []
    for i in range(N_CORES):
        Y = results[i]["Y"]                               # (32, 128, R)
        y = Y.reshape(32, 64, 2, R).transpose(3, 1, 0, 2).reshape(R, DIM)
        outs.append(y)
    return np.concatenate(outs, axis=0).reshape(4, 4096, DIM).astype(np.float32)


def _install_ntff_hook():
    """Provide antenv.axon_hooks.get_axon_ntff_profile_hook via ctypes on
    the baked libaxon_pjrt.so (the agent image lacks the module). Only used
    on the _trace path."""
    import sys, types, ctypes, contextlib
    if "antenv.axon_hooks" in sys.modules:
        return
    try:
        lib = ctypes.CDLL("/opt/axon/libaxon_pjrt.so")
        if not hasattr(lib, "axon_start_nrt_profile"):
            return
    except OSError:
        return
    lib.axon_start_nrt_profile.argtypes = [ctypes.POINTER(ctypes.c_int64),
                                           ctypes.c_size_t]
    lib.axon_start_nrt_profile.restype = ctypes.c_int64
    lib.axon_stop_nrt_profile.argtypes = [ctypes.c_char_p]
    lib.axon_stop_nrt_profile.restype = ctypes.c_int64

    @contextlib.contextmanager
    def _hook(output_dir, device_ids):
        import jax
        jax.devices()
        if device_ids:
            ids = (ctypes.c_int64 * len(device_ids))(*device_ids)
            rc = lib.axon_start_nrt_profile(ids, len(device_ids))
        else:
            rc = lib.axon_start_nrt_profile(None, 0)
        if rc != 0:
            raise RuntimeError(f"axon_start_nrt_profile rc={rc}")
        try:
            yield
        finally:
            n = lib.axon_stop_nrt_profile(str(output_dir).encode())
            print(f"ntff profile: {n} file(s) -> {output_dir}")

    mod = types.ModuleType("antenv.axon_hooks")
    mod.get_axon_ntff_profile_hook = lambda: _hook
    sys.modules["antenv.axon_hooks"] = mod


def kernel(x, H, _trace=False, _loop=0, _cfg=None):
    if _trace:
        _install_ntff_hook()
        from concourse import bass_utils as _bu
        _bu.upload_artifacts = lambda d: d
    nc = _build_bass(MODE, loop=_loop, cfg=_cfg)
    in_maps = _prep_inputs(x, H, MODE)
    res = run_bass_kernel_spmd(nc, in_maps, core_ids=list(range(N_CORES)),
                               trace=_trace)
    out = _unscramble(res.results)
    if _trace:
        return out, res
    return out


# revision 11
# speedup vs baseline: 1.5393x; 1.5269x over previous
"""Trainium2 kernel for nn_HadamardRotation: y = x @ H, H = 4096x4096 Walsh-Hadamard.

Strategy
--------
H4096 = H64 (x) H64 (Kronecker). Writing d = 64*hi + lo, e = 64*hi' + lo':

    y[r, e] = sum_{hi,lo} H64[lo,lo'] * H64[hi,hi'] * x[r, d]

Two matmul stages with 128-wide contraction (block-diagonal I2 (x) H64
weights), separated by an on-chip "corner turn" (SBUF->SBUF DMA partition
shuffle), all operating in the transposed domain (d on partitions, rows on
the free axis). Host does the cheap transposes / index unscrambles; the
device only ever issues contiguous 4KB DMA lines.

FLOPs: 2 * 128/4096 of the naive matmul = 16x reduction.

Data parallel over 8 cores: rows sharded 16384 -> 8 x 2048, weights
replicated.

Layouts (per core, R = 2048 rows):
  xt  DRAM in  (32, 128, R): xt[a, 64*mu+lo, r] = x[r, 128*a + 64*mu + lo]
  B1  (128,128): B1[64*mu+lo, 2*lo'+mu]     = H64[lo, lo']
  B2  (128,128): B2[64*nu+32*mu+a, 2*hi'+nu] = H64[2*a+mu, hi']
  stage A (chunk a): u[p, a, r] = sum_k B1[k, p] xt[a, k, r]
      => u[4c + (2*nu+mu), a] holds (hi = 2a+mu, lo' = 2c+nu)
  corner turn:  v_c[32*t + a, r] = u[4*c + t, a, r]
  stage B (chunk c): Y[c, m, r] = sum_q B2[q, m] v_c[q, r]
      => Y[c, 2*hi'+nu, r] = y[r, 64*hi' + 2*c + nu]

Perf notes (from NTFF profiles): the kernel is DMA-bound. All bulk DMA
(input load, corner turn, output store) is issued from the SP engine so it
lands on qSyncDynamicHW, the only queue striped over all 16 DMA engines
(the scalar/ACT HWDGE queue only gets 4). Every transfer moves 4KB
contiguous lines. Output is written bf16 (host upcasts) to halve the HBM
write. Total DMA = 3 x 16.8MB per core ~= 50MB at 360 GB/s aggregate.
"""

import math
import numpy as np
import ml_dtypes

import concourse.bass as bass
import concourse.mybir as mybir
import concourse.tile as tile
from concourse import bacc
from concourse.bass_utils import run_bass_kernel_spmd

N_CORES = 8
DIM = 4096
R_TOTAL = 4 * 4096          # rows after flattening (4, 4096, DIM)
R = R_TOTAL // N_CORES      # rows per core
N = 512                     # matmul free-dim slab (one PSUM bank of fp32)
NSLICE = R // N             # matmul slices per chunk

MODE = "bf16"

CFG = dict(
    in_eng="sync",
    turn_eng="gpsimd",
    out_eng="sync",
    copy_rot=("vector", "any"),  # engines for psum->sbuf copies, round robin
    xbufs=2, vbufs=2, ybufs=2,
    in_b=2,                      # input chunks per DMA
    out_b=2,                     # output chunks per DMA
    turn_g=4,                    # c-chunks per corner-turn DMA (>=4 spreads
                                 # the turn's per-src-partition descriptor
                                 # chains over 4*turn_g DMA engines)
)


def _walsh_hadamard64():
    h = np.array([[1.0]], dtype=np.float64)
    while h.shape[0] < 64:
        h = np.block([[h, h], [h, -h]]) / math.sqrt(2.0)
    return h.astype(np.float32)


def _build_weights(H64):
    B1 = np.zeros((128, 128), dtype=np.float32)
    b1v = B1.reshape(2, 64, 64, 2)
    for mu in range(2):
        b1v[mu, :, :, mu] = H64
    B2 = np.zeros((128, 128), dtype=np.float32)
    b2v = B2.reshape(2, 2, 32, 64, 2)
    for nu in range(2):
        for mu in range(2):
            b2v[nu, mu, :, :, nu] = H64[mu::2, :]
    return B1, B2


_NC_CACHE = {}


def _build_bass(mode, loop=0, cfg=None):
    cfg = dict(CFG, **(cfg or {}))
    key = (mode, loop, tuple(sorted((k, str(v)) for k, v in cfg.items())))
    if key in _NC_CACHE:
        return _NC_CACHE[key]

    f32 = mybir.dt.float32
    dt_in = mybir.dt.bfloat16 if mode == "bf16" else f32
    dt_out = mybir.dt.bfloat16 if mode == "bf16" else f32

    nc = bacc.Bacc("TRN2", target_bir_lowering=False, debug=False,
                   num_devices=N_CORES)
    xt_d = nc.dram_tensor("xt", [32, 128, R], dt_in, kind="ExternalInput")
    B1_d = nc.dram_tensor("B1", [128, 128], dt_in, kind="ExternalInput")
    B2_d = nc.dram_tensor("B2", [128, 128], dt_in, kind="ExternalInput")
    Y_d = nc.dram_tensor("Y", [32, 128, R], dt_out, kind="ExternalOutput")

    with tile.TileContext(nc) as tc:
        with (
            tc.tile_pool(name="wpool", bufs=1) as wpool,
            tc.tile_pool(name="xpool", bufs=cfg["xbufs"]) as xpool,
            tc.tile_pool(name="upool", bufs=1) as upool,
            tc.tile_pool(name="vpool", bufs=cfg["vbufs"]) as vpool,
            tc.tile_pool(name="ypool", bufs=cfg["ybufs"]) as ypool,
            tc.tile_pool(name="psA", bufs=4, space="PSUM") as psA,
            tc.tile_pool(name="psB", bufs=4, space="PSUM") as psB,
        ):
            B1_sb = wpool.tile([128, 128], dt_in)
            nc.sync.dma_start(B1_sb[:], B1_d[:])
            B2_sb = wpool.tile([128, 128], dt_in)
            nc.sync.dma_start(B2_sb[:], B2_d[:])

            in_eng = getattr(nc, cfg["in_eng"])
            turn_eng = getattr(nc, cfg["turn_eng"])
            out_eng = getattr(nc, cfg["out_eng"])
            rot = cfg["copy_rot"]

            def copy(dst, src, i):
                getattr(nc, rot[i % len(rot)]).tensor_copy(dst, src)

            IB = cfg["in_b"]
            OB = cfg["out_b"]
            TG = cfg["turn_g"]

            def body():
                u_full = upool.tile([128, 32, R], dt_in)

                # ---- phase A: load chunks, stage-A matmuls into u ----
                ci = 0
                for ab in range(32 // IB):
                    xg = xpool.tile([128, IB, R], dt_in)
                    in_eng.dma_start(
                        xg[:], xt_d[IB * ab:IB * (ab + 1)].transpose([1, 0, 2]))
                    for j in range(IB):
                        a = IB * ab + j
                        for s in range(NSLICE):
                            pu = psA.tile([128, N], f32)
                            nc.tensor.matmul(pu[:], B1_sb[:],
                                             xg[:, j, s * N:(s + 1) * N],
                                             start=True, stop=True)
                            copy(u_full[:, a, s * N:(s + 1) * N], pu[:], ci)
                            ci += 1

                # ---- phase B: corner turn + stage-B matmuls + store ----
                # The turn's flat-merged src AP defeats tile's subtile dep
                # analysis; the algorithm has a true all-of-u barrier here
                # anyway, so make it explicit.
                tc.strict_bb_all_engine_barrier()

                PU = u_full.ap[0][0]  # partition stride in elements

                def turn(g):
                    # v_c[32t+a, r] = u[4c+t, a, r], one DMA per c. SBUF APs
                    # need partition dims first, so the src reads 4 partitions
                    # per transfer (hardware HWDGE then stripes it over only 4
                    # DMA engines; SWDGE/gpsimd assigns rings per descriptor).
                    vg = vpool.tile([128, TG, R], dt_in)
                    for cj in range(TG):
                        c = TG * g + cj
                        src = bass.AP(u_full.tensor, 4 * c * PU,
                                      [[PU, 4], [R, 32], [1, R]])
                        turn_eng.dma_start(vg[:, cj, :], src)
                    return vg

                def stageB(g, vg, ci):
                    for cb in range(TG // OB):
                        yb = ypool.tile([128, OB, R], dt_out)
                        for j in range(OB):
                            cj = cb * OB + j
                            for s in range(NSLICE):
                                py = psB.tile([128, N], f32)
                                nc.tensor.matmul(py[:], B2_sb[:],
                                                 vg[:, cj, s * N:(s + 1) * N],
                                                 start=True, stop=True)
                                copy(yb[:, j, s * N:(s + 1) * N], py[:], ci)
                                ci += 1
                        c0 = TG * g + cb * OB
                        out_eng.dma_start(
                            Y_d[c0:c0 + OB].transpose([1, 0, 2]), yb[:])
                    return ci

                pend = None
                for g in range(32 // TG):
                    vg = turn(g)
                    if pend is not None:
                        ci = stageB(*pend, ci)
                    pend = (g, vg)
                ci = stageB(*pend, ci)

            if loop:
                with tc.For_i(0, loop, 1):
                    body()
            else:
                body()

    nc.compile()
    _NC_CACHE[key] = nc
    return nc


def _prep_inputs(x, H, mode):
    np_in = ml_dtypes.bfloat16 if mode == "bf16" else np.float32
    H64 = (np.asarray(H, dtype=np.float32)[::64, ::64] * 8.0).astype(np.float32)
    B1, B2 = _build_weights(H64)
    B1 = B1.astype(np_in)
    B2 = B2.astype(np_in)
    xf = np.asarray(x, dtype=np.float32).reshape(R_TOTAL, DIM)
    in_maps = []
    for i in range(N_CORES):
        shard = xf[i * R:(i + 1) * R]                     # (R, DIM)
        xt = np.ascontiguousarray(shard.T, dtype=np_in)   # (DIM, R)
        xt = xt.reshape(32, 128, R)
        in_maps.append({"xt": xt, "B1": B1, "B2": B2})
    return in_maps


def _unscramble(results):
    outs = []
    for i in range(N_CORES):
        Y = results[i]["Y"]                               # (32, 128, R)
        y = Y.reshape(32, 64, 2, R).transpose(3, 1, 0, 2).reshape(R, DIM)
        outs.append(y)
    return np.concatenate(outs, axis=0).reshape(4, 4096, DIM).astype(np.float32)


def _install_ntff_hook():
    """Provide antenv.axon_hooks.get_axon_ntff_profile_hook via ctypes on
    the baked libaxon_pjrt.so (the agent image lacks the module). Only used
    on the _trace path."""
    import sys, types, ctypes, contextlib
    if "antenv.axon_hooks" in sys.modules:
        return
    try:
        lib = ctypes.CDLL("/opt/axon/libaxon_pjrt.so")
        if not hasattr(lib, "axon_start_nrt_profile"):
            return
    except OSError:
        return
    lib.axon_start_nrt_profile.argtypes = [ctypes.POINTER(ctypes.c_int64),
                                           ctypes.c_size_t]
    lib.axon_start_nrt_profile.restype = ctypes.c_int64
    lib.axon_stop_nrt_profile.argtypes = [ctypes.c_char_p]
    lib.axon_stop_nrt_profile.restype = ctypes.c_int64

    @contextlib.contextmanager
    def _hook(output_dir, device_ids):
        import jax
        jax.devices()
        if device_ids:
            ids = (ctypes.c_int64 * len(device_ids))(*device_ids)
            rc = lib.axon_start_nrt_profile(ids, len(device_ids))
        else:
            rc = lib.axon_start_nrt_profile(None, 0)
        if rc != 0:
            raise RuntimeError(f"axon_start_nrt_profile rc={rc}")
        try:
            yield
        finally:
            n = lib.axon_stop_nrt_profile(str(output_dir).encode())
            print(f"ntff profile: {n} file(s) -> {output_dir}")

    mod = types.ModuleType("antenv.axon_hooks")
    mod.get_axon_ntff_profile_hook = lambda: _hook
    sys.modules["antenv.axon_hooks"] = mod


def kernel(x, H, _trace=False, _loop=0, _cfg=None):
    if _trace:
        _install_ntff_hook()
        from concourse import bass_utils as _bu
        _bu.upload_artifacts = lambda d: d
    nc = _build_bass(MODE, loop=_loop, cfg=_cfg)
    in_maps = _prep_inputs(x, H, MODE)
    res = run_bass_kernel_spmd(nc, in_maps, core_ids=list(range(N_CORES)),
                               trace=_trace)
    out = _unscramble(res.results)
    if _trace:
        return out, res
    return out


# revision 15
# speedup vs baseline: 2.1324x; 1.3853x over previous
"""Trainium2 kernel for nn_HadamardRotation: y = x @ H, H = 4096x4096 Walsh-Hadamard.

Strategy
--------
H4096 = H64 (x) H64 (Kronecker). Writing d = 64*hi + lo, e = 64*hi' + lo':

    y[r, e] = sum_{hi,lo} H64[lo,lo'] * H64[hi,hi'] * x[r, d]

Two matmul stages with 128-wide contraction (block-diagonal I2 (x) H64
weights), separated by an on-chip "corner turn" (SBUF->SBUF DMA partition
shuffle), all operating in the transposed domain (d on partitions, rows on
the free axis). Host does the cheap transposes / index unscrambles; the
device only ever issues contiguous 4KB DMA lines.

FLOPs: 2 * 128/4096 of the naive matmul = 16x reduction.

Data parallel over 8 cores: rows sharded 16384 -> 8 x 2048, weights
replicated.

Layouts (per core, R = 2048 rows):
  xt  DRAM in  (32, 128, R): xt[a, 64*mu+lo, r] = x[r, 128*a + 64*mu + lo]
  B1  (128,128): B1[64*mu+lo, 2*lo'+mu]     = H64[lo, lo']
  B2  (128,128): B2[64*nu+32*mu+a, 2*hi'+nu] = H64[2*a+mu, hi']
  stage A (chunk a): u[p, a, r] = sum_k B1[k, p] xt[a, k, r]
      => u[4c + (2*nu+mu), a] holds (hi = 2a+mu, lo' = 2c+nu)
  corner turn:  v_c[32*t + a, r] = u[4*c + t, a, r]
  stage B (chunk c): Y[c, m, r] = sum_q B2[q, m] v_c[q, r]
      => Y[c, 2*hi'+nu, r] = y[r, 64*hi' + 2*c + nu]

Perf notes (from NTFF profiles): the kernel is DMA-bound. All bulk DMA
(input load, corner turn, output store) is issued from the SP engine so it
lands on qSyncDynamicHW, the only queue striped over all 16 DMA engines
(the scalar/ACT HWDGE queue only gets 4). Every transfer moves 4KB
contiguous lines. Output is written bf16 (host upcasts) to halve the HBM
write. Total DMA = 3 x 16.8MB per core ~= 50MB at 360 GB/s aggregate.
"""

import math
import numpy as np
import ml_dtypes

import concourse.bass as bass
import concourse.mybir as mybir
import concourse.tile as tile
from concourse import bacc
from concourse.bass_utils import run_bass_kernel_spmd

N_CORES = 8
DIM = 4096
R_TOTAL = 4 * 4096          # rows after flattening (4, 4096, DIM)
R = R_TOTAL // N_CORES      # rows per core
N = 512                     # matmul free-dim slab (one PSUM bank of fp32)
NSLICE = R // N             # matmul slices per chunk

MODE = "bf16"

CFG = dict(
    in_eng="sync",
    hop1_eng="sync",
    hop2_eng="sync",
    out_eng="sync",
    copy_rot=("vector", "any"),  # engines for psum->sbuf copies, round robin
    xbufs=2, ubufs=4, vbufs=2, ybufs=2,
    in_b=2,                      # input chunks per DMA
    out_b=2,                     # output chunks per DMA
    turn_g=8,                    # c-chunks per hop-2 gather
)


def _walsh_hadamard64():
    h = np.array([[1.0]], dtype=np.float64)
    while h.shape[0] < 64:
        h = np.block([[h, h], [h, -h]]) / math.sqrt(2.0)
    return h.astype(np.float32)


def _build_weights(H64):
    B1 = np.zeros((128, 128), dtype=np.float32)
    b1v = B1.reshape(2, 64, 64, 2)
    for mu in range(2):
        b1v[mu, :, :, mu] = H64
    B2 = np.zeros((128, 128), dtype=np.float32)
    b2v = B2.reshape(2, 2, 32, 64, 2)
    for nu in range(2):
        for mu in range(2):
            b2v[nu, mu, :, :, nu] = H64[mu::2, :]
    return B1, B2


_NC_CACHE = {}


def _build_bass(mode, loop=0, cfg=None):
    cfg = dict(CFG, **(cfg or {}))
    key = (mode, loop, tuple(sorted((k, str(v)) for k, v in cfg.items())))
    if key in _NC_CACHE:
        return _NC_CACHE[key]

    f32 = mybir.dt.float32
    dt_in = mybir.dt.bfloat16 if mode == "bf16" else f32
    dt_out = mybir.dt.bfloat16 if mode == "bf16" else f32

    nc = bacc.Bacc("TRN2", target_bir_lowering=False, debug=False,
                   num_devices=N_CORES)
    xt_d = nc.dram_tensor("xt", [32, 128, R], dt_in, kind="ExternalInput")
    B1_d = nc.dram_tensor("B1", [128, 128], dt_in, kind="ExternalInput")
    B2_d = nc.dram_tensor("B2", [128, 128], dt_in, kind="ExternalInput")
    Y_d = nc.dram_tensor("Y", [32, 128, R], dt_out, kind="ExternalOutput")
    # corner-turn scratch, layout [c, t, a, r]: T[c,t,a,r] = u[4c+t, a, r].
    # Routing the turn through DRAM keeps every DMA a clean 128-partition
    # HWDGE transfer (stripes over all 16 DMA engines with 4KB lines);
    # direct SBUF->SBUF turns are limited to 4 engines (HWDGE, one
    # descriptor chain per source partition) or ~6.3 GB/s/engine (SWDGE).
    T_d = nc.dram_tensor("Tsc", [32, 4, 32, R], dt_in, kind="Internal")

    with tile.TileContext(nc) as tc:
        with (
            tc.tile_pool(name="wpool", bufs=1) as wpool,
            tc.tile_pool(name="xpool", bufs=cfg["xbufs"]) as xpool,
            tc.tile_pool(name="upool", bufs=cfg["ubufs"]) as upool,
            tc.tile_pool(name="vpool", bufs=cfg["vbufs"]) as vpool,
            tc.tile_pool(name="ypool", bufs=cfg["ybufs"]) as ypool,
            tc.tile_pool(name="psA", bufs=4, space="PSUM") as psA,
            tc.tile_pool(name="psB", bufs=4, space="PSUM") as psB,
        ):
            B1_sb = wpool.tile([128, 128], dt_in)
            nc.sync.dma_start(B1_sb[:], B1_d[:])
            B2_sb = wpool.tile([128, 128], dt_in)
            nc.sync.dma_start(B2_sb[:], B2_d[:])

            in_eng = getattr(nc, cfg["in_eng"])
            hop1_eng = getattr(nc, cfg["hop1_eng"])
            hop2_eng = getattr(nc, cfg["hop2_eng"])
            out_eng = getattr(nc, cfg["out_eng"])
            rot = cfg["copy_rot"]

            def copy(dst, src, i):
                getattr(nc, rot[i % len(rot)]).tensor_copy(dst, src)

            IB = cfg["in_b"]
            OB = cfg["out_b"]
            TG = cfg["turn_g"]

            def body():
                # ---- phase A: load chunks, stage-A matmuls, spill u to T ----
                ci = 0
                hop_pend = []
                for ab in range(32 // IB):
                    xg = xpool.tile([128, IB, R], dt_in)
                    in_eng.dma_start(
                        xg[:], xt_d[IB * ab:IB * (ab + 1)].transpose([1, 0, 2]))
                    # delay hop1 issue one batch so it doesn't head-of-line
                    # block the next input load on the SP queue
                    while hop_pend:
                        a_, ua_ = hop_pend.pop(0)
                        hop1_eng.dma_start(T_d[:, :, a_, :], ua_[:])
                    for j in range(IB):
                        a = IB * ab + j
                        ua = upool.tile([128, R], dt_in)
                        for s in range(NSLICE):
                            pu = psA.tile([128, N], f32)
                            nc.tensor.matmul(pu[:], B1_sb[:],
                                             xg[:, j, s * N:(s + 1) * N],
                                             start=True, stop=True)
                            copy(ua[:, s * N:(s + 1) * N], pu[:], ci)
                            ci += 1
                        hop_pend.append((a, ua))
                for a_, ua_ in hop_pend:
                    hop1_eng.dma_start(T_d[:, :, a_, :], ua_[:])

                # ---- phase B: gather v from T, stage-B matmuls, store ----
                def hop2(g):
                    # v[32t+a, c', r] = T[TG*g+c', t, a, r]; (t, a) flat on
                    # the DRAM side with stride R, so 3-dim gather.
                    vg = vpool.tile([128, TG, R], dt_in)
                    src = bass.AP(T_d, TG * g * 128 * R,
                                  [[R, 128], [128 * R, TG], [1, R]])
                    hop2_eng.dma_start(vg[:], src)
                    return vg

                def stageB(g, vg, ci):
                    for cb in range(TG // OB):
                        yb = ypool.tile([128, OB, R], dt_out)
                        for j in range(OB):
                            cj = cb * OB + j
                            for s in range(NSLICE):
                                py = psB.tile([128, N], f32)
                                nc.tensor.matmul(py[:], B2_sb[:],
                                                 vg[:, cj, s * N:(s + 1) * N],
                                                 start=True, stop=True)
                                copy(yb[:, j, s * N:(s + 1) * N], py[:], ci)
                                ci += 1
                        c0 = TG * g + cb * OB
                        out_eng.dma_start(
                            Y_d[c0:c0 + OB].transpose([1, 0, 2]), yb[:])
                    return ci

                pend = None
                for g in range(32 // TG):
                    vg = hop2(g)
                    if pend is not None:
                        ci = stageB(*pend, ci)
                    pend = (g, vg)
                ci = stageB(*pend, ci)

            if loop:
                with tc.For_i(0, loop, 1):
                    body()
            else:
                body()

    nc.compile()
    _NC_CACHE[key] = nc
    return nc


def _prep_inputs(x, H, mode):
    np_in = ml_dtypes.bfloat16 if mode == "bf16" else np.float32
    H64 = (np.asarray(H, dtype=np.float32)[::64, ::64] * 8.0).astype(np.float32)
    B1, B2 = _build_weights(H64)
    B1 = B1.astype(np_in)
    B2 = B2.astype(np_in)
    xf = np.asarray(x, dtype=np.float32).reshape(R_TOTAL, DIM)
    in_maps = []
    for i in range(N_CORES):
        shard = xf[i * R:(i + 1) * R]                     # (R, DIM)
        xt = np.ascontiguousarray(shard.T, dtype=np_in)   # (DIM, R)
        xt = xt.reshape(32, 128, R)
        in_maps.append({"xt": xt, "B1": B1, "B2": B2})
    return in_maps


def _unscramble(results):
    outs = []
    for i in range(N_CORES):
        Y = results[i]["Y"]                               # (32, 128, R)
        y = Y.reshape(32, 64, 2, R).transpose(3, 1, 0, 2).reshape(R, DIM)
        outs.append(y)
    return np.concatenate(outs, axis=0).reshape(4, 4096, DIM).astype(np.float32)


def _install_ntff_hook():
    """Provide antenv.axon_hooks.get_axon_ntff_profile_hook via ctypes on
    the baked libaxon_pjrt.so (the agent image lacks the module). Only used
    on the _trace path."""
    import sys, types, ctypes, contextlib
    if "antenv.axon_hooks" in sys.modules:
        return
    try:
        lib = ctypes.CDLL("/opt/axon/libaxon_pjrt.so")
        if not hasattr(lib, "axon_start_nrt_profile"):
            return
    except OSError:
        return
    lib.axon_start_nrt_profile.argtypes = [ctypes.POINTER(ctypes.c_int64),
                                           ctypes.c_size_t]
    lib.axon_start_nrt_profile.restype = ctypes.c_int64
    lib.axon_stop_nrt_profile.argtypes = [ctypes.c_char_p]
    lib.axon_stop_nrt_profile.restype = ctypes.c_int64

    @contextlib.contextmanager
    def _hook(output_dir, device_ids):
        import jax
        jax.devices()
        if device_ids:
            ids = (ctypes.c_int64 * len(device_ids))(*device_ids)
            rc = lib.axon_start_nrt_profile(ids, len(device_ids))
        else:
            rc = lib.axon_start_nrt_profile(None, 0)
        if rc != 0:
            raise RuntimeError(f"axon_start_nrt_profile rc={rc}")
        try:
            yield
        finally:
            n = lib.axon_stop_nrt_profile(str(output_dir).encode())
            print(f"ntff profile: {n} file(s) -> {output_dir}")

    mod = types.ModuleType("antenv.axon_hooks")
    mod.get_axon_ntff_profile_hook = lambda: _hook
    sys.modules["antenv.axon_hooks"] = mod


def kernel(x, H, _trace=False, _loop=0, _cfg=None):
    if _trace:
        _install_ntff_hook()
        from concourse import bass_utils as _bu
        _bu.upload_artifacts = lambda d: d
    nc = _build_bass(MODE, loop=_loop, cfg=_cfg)
    in_maps = _prep_inputs(x, H, MODE)
    res = run_bass_kernel_spmd(nc, in_maps, core_ids=list(range(N_CORES)),
                               trace=_trace)
    out = _unscramble(res.results)
    if _trace:
        return out, res
    return out


# revision 17
# speedup vs baseline: 2.2426x; 1.0517x over previous
"""Trainium2 kernel for nn_HadamardRotation: y = x @ H, H = 4096x4096 Walsh-Hadamard.

Strategy
--------
H4096 = H64 (x) H64 (Kronecker). Writing d = 64*hi + lo, e = 64*hi' + lo':

    y[r, e] = sum_{hi,lo} H64[lo,lo'] * H64[hi,hi'] * x[r, d]

Two matmul stages with 128-wide contraction (block-diagonal I2 (x) H64
weights), separated by an on-chip "corner turn" (SBUF->SBUF DMA partition
shuffle), all operating in the transposed domain (d on partitions, rows on
the free axis). Host does the cheap transposes / index unscrambles; the
device only ever issues contiguous 4KB DMA lines.

FLOPs: 2 * 128/4096 of the naive matmul = 16x reduction.

Data parallel over 8 cores: rows sharded 16384 -> 8 x 2048, weights
replicated.

Layouts (per core, R = 2048 rows):
  xt  DRAM in  (32, 128, R): xt[a, 64*mu+lo, r] = x[r, 128*a + 64*mu + lo]
  B1  (128,128): B1[64*mu+lo, 2*lo'+mu]     = H64[lo, lo']
  B2  (128,128): B2[64*nu+32*mu+a, 2*hi'+nu] = H64[2*a+mu, hi']
  stage A (chunk a): u[p, a, r] = sum_k B1[k, p] xt[a, k, r]
      => u[4c + (2*nu+mu), a] holds (hi = 2a+mu, lo' = 2c+nu)
  corner turn:  v_c[32*t + a, r] = u[4*c + t, a, r]
  stage B (chunk c): Y[c, m, r] = sum_q B2[q, m] v_c[q, r]
      => Y[c, 2*hi'+nu, r] = y[r, 64*hi' + 2*c + nu]

Perf notes (from NTFF profiles): the kernel is DMA-bound. All bulk DMA
(input load, corner turn, output store) is issued from the SP engine so it
lands on qSyncDynamicHW, the only queue striped over all 16 DMA engines
(the scalar/ACT HWDGE queue only gets 4). Every transfer moves 4KB
contiguous lines. Output is written bf16 (host upcasts) to halve the HBM
write. Total DMA = 3 x 16.8MB per core ~= 50MB at 360 GB/s aggregate.
"""

import math
import numpy as np
import ml_dtypes

import concourse.bass as bass
import concourse.mybir as mybir
import concourse.tile as tile
from concourse import bacc
from concourse.bass_utils import run_bass_kernel_spmd

N_CORES = 8
DIM = 4096
R_TOTAL = 4 * 4096          # rows after flattening (4, 4096, DIM)
R = R_TOTAL // N_CORES      # rows per core
N = 512                     # matmul free-dim slab (one PSUM bank of fp32)
NSLICE = R // N             # matmul slices per chunk

MODE = "bf16"

CFG = dict(
    in_eng="sync",
    hop1_eng="sync",
    hop2_eng="sync",
    out_eng="sync",
    copy_rot=("vector", "any"),  # engines for psum->sbuf copies, round robin
    xbufs=3, ubufs=3, vbufs=3, ybufs=3,
    in_b=2,                      # input chunks per DMA (also hop-1 batch)
    out_b=2,                     # output chunks per DMA
    turn_g=4,                    # c-chunks per hop-2 gather
)


def _walsh_hadamard64():
    h = np.array([[1.0]], dtype=np.float64)
    while h.shape[0] < 64:
        h = np.block([[h, h], [h, -h]]) / math.sqrt(2.0)
    return h.astype(np.float32)


def _build_weights(H64):
    B1 = np.zeros((128, 128), dtype=np.float32)
    b1v = B1.reshape(2, 64, 64, 2)
    for mu in range(2):
        b1v[mu, :, :, mu] = H64
    B2 = np.zeros((128, 128), dtype=np.float32)
    b2v = B2.reshape(2, 2, 32, 64, 2)
    for nu in range(2):
        for mu in range(2):
            b2v[nu, mu, :, :, nu] = H64[mu::2, :]
    return B1, B2


_NC_CACHE = {}


def _build_bass(mode, loop=0, cfg=None):
    cfg = dict(CFG, **(cfg or {}))
    key = (mode, loop, tuple(sorted((k, str(v)) for k, v in cfg.items())))
    if key in _NC_CACHE:
        return _NC_CACHE[key]

    f32 = mybir.dt.float32
    dt_in = mybir.dt.bfloat16 if mode == "bf16" else f32
    dt_out = mybir.dt.bfloat16 if mode == "bf16" else f32

    nc = bacc.Bacc("TRN2", target_bir_lowering=False, debug=False,
                   num_devices=N_CORES)
    xt_d = nc.dram_tensor("xt", [32, 128, R], dt_in, kind="ExternalInput")
    B1_d = nc.dram_tensor("B1", [128, 128], dt_in, kind="ExternalInput")
    B2_d = nc.dram_tensor("B2", [128, 128], dt_in, kind="ExternalInput")
    Y_d = nc.dram_tensor("Y", [32, 128, R], dt_out, kind="ExternalOutput")
    # corner-turn scratch, layout [c, t, a, r]: T[c,t,a,r] = u[4c+t, a, r].
    # Routing the turn through DRAM keeps every DMA a clean 128-partition
    # HWDGE transfer (stripes over all 16 DMA engines with 4KB lines);
    # direct SBUF->SBUF turns are limited to 4 engines (HWDGE, one
    # descriptor chain per source partition) or ~6.3 GB/s/engine (SWDGE).
    T_d = nc.dram_tensor("Tsc", [32, 4, 32, R], dt_in, kind="Internal")

    with tile.TileContext(nc) as tc:
        with (
            tc.tile_pool(name="wpool", bufs=1) as wpool,
            tc.tile_pool(name="xpool", bufs=cfg["xbufs"]) as xpool,
            tc.tile_pool(name="upool", bufs=cfg["ubufs"]) as upool,
            tc.tile_pool(name="vpool", bufs=cfg["vbufs"]) as vpool,
            tc.tile_pool(name="ypool", bufs=cfg["ybufs"]) as ypool,
            tc.tile_pool(name="psA", bufs=4, space="PSUM") as psA,
            tc.tile_pool(name="psB", bufs=4, space="PSUM") as psB,
        ):
            B1_sb = wpool.tile([128, 128], dt_in)
            nc.sync.dma_start(B1_sb[:], B1_d[:])
            B2_sb = wpool.tile([128, 128], dt_in)
            nc.sync.dma_start(B2_sb[:], B2_d[:])

            in_eng = getattr(nc, cfg["in_eng"])
            hop1_eng = getattr(nc, cfg["hop1_eng"])
            hop2_eng = getattr(nc, cfg["hop2_eng"])
            out_eng = getattr(nc, cfg["out_eng"])
            rot = cfg["copy_rot"]

            def copy(dst, src, i):
                getattr(nc, rot[i % len(rot)]).tensor_copy(dst, src)

            IB = cfg["in_b"]
            OB = cfg["out_b"]
            TG = cfg["turn_g"]

            def body():
                # ---- phase A: load chunks, stage-A matmuls, spill u to T ----
                ci = 0
                hop_pend = []
                for ab in range(32 // IB):
                    xg = xpool.tile([128, IB, R], dt_in)
                    in_eng.dma_start(
                        xg[:], xt_d[IB * ab:IB * (ab + 1)].transpose([1, 0, 2]))
                    # delay hop1 issue one batch so it doesn't head-of-line
                    # block the next input load on the SP queue
                    while hop_pend:
                        ab_, ua_ = hop_pend.pop(0)
                        hop1_eng.dma_start(
                            T_d[:, :, IB * ab_:IB * (ab_ + 1), :], ua_[:])
                    ua = upool.tile([128, IB, R], dt_in)
                    for j in range(IB):
                        a = IB * ab + j
                        for s in range(NSLICE):
                            pu = psA.tile([128, N], f32)
                            nc.tensor.matmul(pu[:], B1_sb[:],
                                             xg[:, j, s * N:(s + 1) * N],
                                             start=True, stop=True)
                            copy(ua[:, j, s * N:(s + 1) * N], pu[:], ci)
                            ci += 1
                    hop_pend.append((ab, ua))
                for ab_, ua_ in hop_pend:
                    hop1_eng.dma_start(
                        T_d[:, :, IB * ab_:IB * (ab_ + 1), :], ua_[:])

                # ---- phase B: gather v from T, stage-B matmuls, store ----
                def hop2(g):
                    # v[32t+a, c', r] = T[TG*g+c', t, a, r]; (t, a) flat on
                    # the DRAM side with stride R, so 3-dim gather.
                    vg = vpool.tile([128, TG, R], dt_in)
                    src = bass.AP(T_d, TG * g * 128 * R,
                                  [[R, 128], [128 * R, TG], [1, R]])
                    hop2_eng.dma_start(vg[:], src)
                    return vg

                def stageB(g, vg, ci):
                    for cb in range(TG // OB):
                        yb = ypool.tile([128, OB, R], dt_out)
                        for j in range(OB):
                            cj = cb * OB + j
                            for s in range(NSLICE):
                                py = psB.tile([128, N], f32)
                                nc.tensor.matmul(py[:], B2_sb[:],
                                                 vg[:, cj, s * N:(s + 1) * N],
                                                 start=True, stop=True)
                                copy(yb[:, j, s * N:(s + 1) * N], py[:], ci)
                                ci += 1
                        c0 = TG * g + cb * OB
                        out_eng.dma_start(
                            Y_d[c0:c0 + OB].transpose([1, 0, 2]), yb[:])
                    return ci

                pend = None
                for g in range(32 // TG):
                    vg = hop2(g)
                    if pend is not None:
                        ci = stageB(*pend, ci)
                    pend = (g, vg)
                ci = stageB(*pend, ci)

            if loop:
                with tc.For_i(0, loop, 1):
                    body()
            else:
                body()

    nc.compile()
    _NC_CACHE[key] = nc
    return nc


def _prep_inputs(x, H, mode):
    np_in = ml_dtypes.bfloat16 if mode == "bf16" else np.float32
    H64 = (np.asarray(H, dtype=np.float32)[::64, ::64] * 8.0).astype(np.float32)
    B1, B2 = _build_weights(H64)
    B1 = B1.astype(np_in)
    B2 = B2.astype(np_in)
    xf = np.asarray(x, dtype=np.float32).reshape(R_TOTAL, DIM)
    in_maps = []
    for i in range(N_CORES):
        shard = xf[i * R:(i + 1) * R]                     # (R, DIM)
        xt = np.ascontiguousarray(shard.T, dtype=np_in)   # (DIM, R)
        xt = xt.reshape(32, 128, R)
        in_maps.append({"xt": xt, "B1": B1, "B2": B2})
    return in_maps


def _unscramble(results):
    outs = []
    for i in range(N_CORES):
        Y = results[i]["Y"]                               # (32, 128, R)
        y = Y.reshape(32, 64, 2, R).transpose(3, 1, 0, 2).reshape(R, DIM)
        outs.append(y)
    return np.concatenate(outs, axis=0).reshape(4, 4096, DIM).astype(np.float32)


def _install_ntff_hook():
    """Provide antenv.axon_hooks.get_axon_ntff_profile_hook via ctypes on
    the baked libaxon_pjrt.so (the agent image lacks the module). Only used
    on the _trace path."""
    import sys, types, ctypes, contextlib
    if "antenv.axon_hooks" in sys.modules:
        return
    try:
        lib = ctypes.CDLL("/opt/axon/libaxon_pjrt.so")
        if not hasattr(lib, "axon_start_nrt_profile"):
            return
    except OSError:
        return
    lib.axon_start_nrt_profile.argtypes = [ctypes.POINTER(ctypes.c_int64),
                                           ctypes.c_size_t]
    lib.axon_start_nrt_profile.restype = ctypes.c_int64
    lib.axon_stop_nrt_profile.argtypes = [ctypes.c_char_p]
    lib.axon_stop_nrt_profile.restype = ctypes.c_int64

    @contextlib.contextmanager
    def _hook(output_dir, device_ids):
        import jax
        jax.devices()
        if device_ids:
            ids = (ctypes.c_int64 * len(device_ids))(*device_ids)
            rc = lib.axon_start_nrt_profile(ids, len(device_ids))
        else:
            rc = lib.axon_start_nrt_profile(None, 0)
        if rc != 0:
            raise RuntimeError(f"axon_start_nrt_profile rc={rc}")
        try:
            yield
        finally:
            n = lib.axon_stop_nrt_profile(str(output_dir).encode())
            print(f"ntff profile: {n} file(s) -> {output_dir}")

    mod = types.ModuleType("antenv.axon_hooks")
    mod.get_axon_ntff_profile_hook = lambda: _hook
    sys.modules["antenv.axon_hooks"] = mod


def kernel(x, H, _trace=False, _loop=0, _cfg=None):
    if _trace:
        _install_ntff_hook()
        from concourse import bass_utils as _bu
        _bu.upload_artifacts = lambda d: d
    nc = _build_bass(MODE, loop=_loop, cfg=_cfg)
    in_maps = _prep_inputs(x, H, MODE)
    res = run_bass_kernel_spmd(nc, in_maps, core_ids=list(range(N_CORES)),
                               trace=_trace)
    out = _unscramble(res.results)
    if _trace:
        return out, res
    return out
